# revision 15
# baseline (speedup 1.0000x reference)
"""Causal multi-head attention block (QKV proj -> causal attention -> out proj)
for Trainium2, sharded over 8 NeuronCores.

Sharding: tensor/data hybrid. Core c handles batch b = c//2 and half the heads
(g = c%2, 8 of 16 heads). Per core:
  - QKV projection for its 8 heads with fp32r matmuls (x^T resident in SBUF)
  - flash-style causal attention in S^T = K @ Q^T layout: exp on ScalarE,
    P^T (fp16) @ V_aug (fp16, ones column appended -> row sums for free)
  - normalize by DVE reciprocal of the fused row sums
  - DMA-xbar transpose of O, fp16 output projection -> partial y [T, C]
Host: y[b] = partial[2b] + partial[2b+1] (+ bias terms, see below).

Biases: b_attn Q/K slices are added on-device (per-partition add fused into
the PSUM->SBUF copies). The V-bias and b_proj contributions are exact row
vectors on the output (rows of softmax sum to 1): y += (b_v @ w_proj + b_proj),
added on host during the unshard.
"""

import math

import numpy as np

import concourse.bass as bass
import concourse.mybir as mybir
import concourse.tile as tile
from concourse import bacc
from concourse.bass_utils import run_bass_kernel_spmd

B, T, C = 4, 2048, 1024
NH, HD = 16, 64
NCORES = 8
HPC = NH // 2          # heads per core = 8
CPC = HPC * HD         # channels per core = 512
P = 128                # partitions
NT = T // P            # 16 t-tiles of 128
NCB = C // P           # 8 contraction blocks
NPAIR = HPC // 2       # 4 head pairs
QW = 512               # q-tile width
NQT = T // QW          # 4 q-tiles

F32 = mybir.dt.float32
F32R = mybir.dt.float32r
F16 = mybir.dt.float16
SCALE = HD ** -0.5


def _r(ap):
    return ap.bitcast(F32R)


def build_kernel(loop_n: int = 1):
    nc = bacc.Bacc("TRN2", target_bir_lowering=False, debug=False)
    xT = nc.dram_tensor("xT", [C, T], F32R, kind="ExternalInput").ap()
    wq = nc.dram_tensor("wq", [C, CPC], F32R, kind="ExternalInput").ap()
    wk = nc.dram_tensor("wk", [C, CPC], F32R, kind="ExternalInput").ap()
    wv = nc.dram_tensor("wv", [C, CPC], F32R, kind="ExternalInput").ap()
    wp = nc.dram_tensor("wp", [CPC, C], F32, kind="ExternalInput").ap()
    qb = nc.dram_tensor("qb", [CPC], F32, kind="ExternalInput").ap()
    kb = nc.dram_tensor("kb", [CPC], F32, kind="ExternalInput").ap()
    maskT = nc.dram_tensor("maskT", [P, P], F16, kind="ExternalInput").ap()
    iden = nc.dram_tensor("iden", [P, P], F16, kind="ExternalInput").ap()
    y = nc.dram_tensor("y", [T, C], F32, kind="ExternalOutput").ap()

    with tile.TileContext(nc) as tc:
        if loop_n == 1:
            _body(tc, nc, xT, wq, wk, wv, wp, qb, kb, maskT, iden, y)
        else:
            with tc.For_i(0, loop_n, 1):
                _body(tc, nc, xT, wq, wk, wv, wp, qb, kb, maskT, iden, y)
    nc.compile()
    return nc


def _body(tc, nc, xT, wq, wk, wv, wp, qb, kb, maskT, iden, y):
    from contextlib import ExitStack

    ctx = ExitStack()
    with ctx:
        const = ctx.enter_context(tc.tile_pool(name="const", bufs=1))
        xt_pool = ctx.enter_context(tc.tile_pool(name="xt", bufs=NCB))
        v_pool = ctx.enter_context(tc.tile_pool(name="vp", bufs=NT))
        wqk_pool = ctx.enter_context(tc.tile_pool(name="wqk", bufs=2))
        qtkt_pool = ctx.enter_context(tc.tile_pool(name="qtkt", bufs=2))
        bias_pool = ctx.enter_context(tc.tile_pool(name="biasp", bufs=2))
        pt_pool = ctx.enter_context(tc.tile_pool(name="ptp", bufs=9))
        osb_pool = ctx.enter_context(tc.tile_pool(name="osb", bufs=2))
        ot_pool = ctx.enter_context(tc.tile_pool(name="otp", bufs=NPAIR))
        r_pool = ctx.enter_context(tc.tile_pool(name="rp", bufs=4))
        wp_pool = ctx.enter_context(tc.tile_pool(name="wpp", bufs=NPAIR))
        y_pool = ctx.enter_context(tc.tile_pool(name="yp", bufs=2))
        mm_ps = ctx.enter_context(tc.tile_pool(name="mmps", bufs=2, space="PSUM"))
        s_ps = ctx.enter_context(tc.tile_pool(name="sps", bufs=2, space="PSUM"))
        o_ps = ctx.enter_context(tc.tile_pool(name="ops", bufs=2, space="PSUM"))

        mask_sb = const.tile([P, P], F16)
        nc.sync.dma_start(out=mask_sb, in_=maskT)
        iden_sb = const.tile([P, P], F16)
        nc.sync.dma_start(out=iden_sb, in_=iden)

        # ---- resident x^T ----
        xt_sb = []
        for i in range(NCB):
            t_ = xt_pool.tile([P, T], F32R, name=f"xt{i}", tag="xt")
            nc.sync.dma_start(out=t_, in_=xT[P * i:P * (i + 1), :])
            xt_sb.append(t_)

        # ---- phase V: V for all 8 heads, fp32r matmuls ----
        v_sb = []
        with tc.tile_pool(name="wvp", bufs=NCB) as wv_pool:
            wv_sb = []
            for i in range(NCB):
                t_ = wv_pool.tile([P, CPC], F32R, name=f"wv{i}", tag="wv")
                nc.sync.dma_start(out=t_, in_=wv[P * i:P * (i + 1), :])
                wv_sb.append(t_)
            for t in range(NT):
                vps = mm_ps.tile([P, CPC], F32, name=f"vps{t}", tag="mm")
                for i in range(NCB):
                    nc.tensor.matmul(
                        vps, (xt_sb[i][:, P * t:P * (t + 1)]), (wv_sb[i]),
                        start=(i == 0), stop=(i == NCB - 1))
                vt = v_pool.tile([P, HPC, HD + 1], F16, name=f"v{t}", tag="v")
                nc.vector.memset(vt[:, :, HD], 1.0)
                nc.vector.tensor_copy(
                    out=vt[:, :, 0:HD],
                    in_=vps.rearrange("p (h d) -> p h d", h=HPC))
                v_sb.append(vt)

        # ---- per head-pair: QT/KT projection + attention ----
        ot_sb = []
        for p in range(NPAIR):
            wq_sb = wqk_pool.tile([P, NCB, P], F32R, name=f"wq{p}", tag="wq")
            wk_sb = wqk_pool.tile([P, NCB, P], F32R, name=f"wk{p}", tag="wk")
            for i in range(NCB):
                nc.sync.dma_start(
                    out=wq_sb[:, i, :],
                    in_=wq[P * i:P * (i + 1), P * p:P * (p + 1)])
                nc.sync.dma_start(
                    out=wk_sb[:, i, :],
                    in_=wk[P * i:P * (i + 1), P * p:P * (p + 1)])
            qb_sb = bias_pool.tile([P, 1], F32, name=f"qb{p}", tag="qb")
            kb_sb = bias_pool.tile([P, 1], F32, name=f"kb{p}", tag="kb")
            nc.sync.dma_start(out=qb_sb, in_=qb[P * p:P * (p + 1)].unsqueeze(1))
            nc.sync.dma_start(out=kb_sb, in_=kb[P * p:P * (p + 1)].unsqueeze(1))

            qt_sb = qtkt_pool.tile([P, T], F32R, name=f"qt{p}", tag="qt")
            kt_sb = qtkt_pool.tile([P, T], F32R, name=f"kt{p}", tag="kt")
            for tq in range(NQT):
                qps = mm_ps.tile([P, QW], F32, name=f"qps{p}{tq}", tag="mm")
                for i in range(NCB):
                    nc.tensor.matmul(
                        qps, (wq_sb[:, i, :]),
                        (xt_sb[i][:, QW * tq:QW * (tq + 1)]),
                        start=(i == 0), stop=(i == NCB - 1))
                nc.vector.tensor_scalar_add(
                    qt_sb[:, QW * tq:QW * (tq + 1)], qps, qb_sb)
                kps = mm_ps.tile([P, QW], F32, name=f"kps{p}{tq}", tag="mm")
                for i in range(NCB):
                    nc.tensor.matmul(
                        kps, (wk_sb[:, i, :]),
                        (xt_sb[i][:, QW * tq:QW * (tq + 1)]),
                        start=(i == 0), stop=(i == NCB - 1))
                nc.vector.tensor_scalar_add(
                    kt_sb[:, QW * tq:QW * (tq + 1)], kps, kb_sb)

            o_sb = osb_pool.tile([P, T], F16, name=f"o{p}", tag="o")
            for hl in range(2):
                hh = 2 * p + hl
                dlo, dhi = HD * hl, HD * (hl + 1)
                for qt_i in range(NQT):
                    ops_ = o_ps.tile([P, 4 * (HD + 1)], F32,
                                     name=f"o{p}{hl}{qt_i}", tag="o")
                    nkt = 4 * qt_i + 4
                    pts = []
                    for k0 in range(0, nkt, 2):
                        sps = s_ps.tile([P, 2 * QW], F32,
                                        name=f"s{p}{hl}{qt_i}{k0}", tag="s")
                        for u in range(2):
                            k = k0 + u
                            nc.tensor.matmul(
                                sps[:, QW * u:QW * (u + 1)],
                                (kt_sb[dlo:dhi, P * k:P * (k + 1)]),
                                (qt_sb[dlo:dhi, QW * qt_i:QW * (qt_i + 1)]),
                                start=True, stop=True)
                        pt = pt_pool.tile([P, 2 * QW], F16,
                                          name=f"pt{p}{hl}{qt_i}{k0}", tag="pt")
                        nc.scalar.activation(
                            out=pt, in_=sps,
                            func=mybir.ActivationFunctionType.Exp, scale=SCALE)
                        for u in range(2):
                            k = k0 + u
                            for s in range(4):
                                gs = 4 * qt_i + s
                                if gs == k:
                                    sl = pt[:, QW * u + P * s:QW * u + P * (s + 1)]
                                    nc.vector.tensor_mul(sl, sl, mask_sb)
                        pts.append(pt)
                    # PV: one open accumulation group per PSUM bank at a time
                    # (start=True marks the whole 2KB zero region pending).
                    for s in range(4):
                        gs = 4 * qt_i + s
                        for k in range(gs + 1):
                            nc.tensor.matmul(
                                ops_[:, (HD + 1) * s:(HD + 1) * (s + 1)],
                                pts[k // 2][:, QW * (k % 2) + P * s:
                                            QW * (k % 2) + P * (s + 1)],
                                v_sb[k][:, hh, :],
                                start=(k == 0), stop=(k == gs))
                    r_ = r_pool.tile([P, 4], F32, name=f"r{p}{hl}{qt_i}", tag="r")
                    nc.vector.reciprocal(
                        r_, ops_.rearrange("p (s c) -> p s c", c=HD + 1)[:, :, HD])
                    out_ap = o_sb[:, QW * qt_i:QW * (qt_i + 1)].rearrange(
                        "p (s h d) -> p s h d", s=4, h=2)[:, :, hl, :]
                    nc.vector.tensor_mul(
                        out_ap,
                        ops_.rearrange("p (s c) -> p s c", c=HD + 1)[:, :, 0:HD],
                        r_.unsqueeze(2).broadcast_to((P, 4, HD)))
            ot = ot_pool.tile([P, T], F16, name=f"ot{p}", tag="ot")
            for tq in range(NQT):
                tp = s_ps.tile([P, QW], F16, name=f"tp{p}{tq}", tag="s")
                for j in range(4):
                    gs = 4 * tq + j
                    nc.tensor.transpose(
                        tp[:, P * j:P * (j + 1)],
                        o_sb[:, P * gs:P * (gs + 1)], iden_sb)
                nc.vector.tensor_copy(
                    out=ot[:, QW * tq:QW * (tq + 1)], in_=tp)
            ot_sb.append(ot)

        # ---- output projection (fp16) ----
        wp16 = []
        for p in range(NPAIR):
            wps = wp_pool.tile([P, C], F32, name=f"wps{p}", tag="wps", bufs=1)
            nc.sync.dma_start(out=wps, in_=wp[P * p:P * (p + 1), :])
            w16 = wp_pool.tile([P, C], F16, name=f"wp16{p}", tag="wp16")
            nc.vector.tensor_copy(out=w16, in_=wps)
            wp16.append(w16)
        for t in range(NT):
            ysb = y_pool.tile([P, C], F32, name=f"y{t}", tag="y")
            for n2 in range(2):
                yps = mm_ps.tile([P, QW], F32, name=f"yps{t}{n2}", tag="mm")
                for p in range(NPAIR):
                    nc.tensor.matmul(
                        yps, ot_sb[p][:, P * t:P * (t + 1)],
                        wp16[p][:, QW * n2:QW * (n2 + 1)],
                        start=(p == 0), stop=(p == NPAIR - 1))
                if n2 == 0:
                    nc.vector.tensor_copy(out=ysb[:, QW * n2:QW * (n2 + 1)], in_=yps)
                else:
                    nc.scalar.copy(out=ysb[:, QW * n2:QW * (n2 + 1)], in_=yps)
            nc.sync.dma_start(out=y[P * t:P * (t + 1), :], in_=ysb)


def _prep_inputs(x, w_attn, b_attn, w_proj):
    """Per-core input maps."""
    in_maps = []
    for c in range(NCORES):
        b = c // 2
        g = c % 2
        qs = slice(CPC * g, CPC * (g + 1))
        ks = slice(C + CPC * g, C + CPC * (g + 1))
        vs = slice(2 * C + CPC * g, 2 * C + CPC * (g + 1))
        in_maps.append({
            "xT": np.ascontiguousarray(x[b].T),
            "wq": np.ascontiguousarray(w_attn[:, qs]),
            "wk": np.ascontiguousarray(w_attn[:, ks]),
            "wv": np.ascontiguousarray(w_attn[:, vs]),
            "wp": np.ascontiguousarray(w_proj[CPC * g:CPC * (g + 1), :]),
            "qb": np.ascontiguousarray(b_attn[qs]),
            "kb": np.ascontiguousarray(b_attn[ks]),
            "maskT": np.triu(np.ones((P, P), dtype=np.float16)),
            "iden": np.eye(P, dtype=np.float16),
        })
    return in_maps


_CACHED_NC = None


def kernel(x, w_attn, b_attn, w_proj, b_proj):
    global _CACHED_NC
    x = np.asarray(x, dtype=np.float32)
    w_attn = np.asarray(w_attn, dtype=np.float32)
    b_attn = np.asarray(b_attn, dtype=np.float32)
    w_proj = np.asarray(w_proj, dtype=np.float32)
    b_proj = np.asarray(b_proj, dtype=np.float32)

    if _CACHED_NC is None:
        _CACHED_NC = build_kernel(loop_n=1)
    nc = _CACHED_NC
    in_maps = _prep_inputs(x, w_attn, b_attn, w_proj)
    res = run_bass_kernel_spmd(nc, in_maps, core_ids=list(range(NCORES)),
                               trace=False)
    out = np.empty((B, T, C), dtype=np.float32)
    # exact row-vector bias contribution: rows of softmax sum to 1
    for b in range(B):
        acc = res.results[2 * b]["y"] + res.results[2 * b + 1]["y"]
        out[b] = acc
    bias_row = b_attn[2 * C:3 * C] @ w_proj + b_proj
    out += bias_row[None, None, :]
    return out
